# revision 1
# baseline (speedup 1.0000x reference)
"""Trainium2 Bass kernel for nn_CumulativeFlattenedLinear (segment_reduce).

Computation: per window of S=64 timesteps, per-timestep C->O linear projection
(weights zero for the first n_discard steps) followed by a causal cumsum within
the window, plus bias.

Strategy (data-parallel over batch, 1 batch element per core):
  - Reformulate per 8-step sub-block u: a triangular-masked "intra" matmul plus
    a "prefix" matmul whose target axis is the later sub-blocks; both share a
    transposed-x stationary and are issued as ONE stacked N=256 fp32r matmul
    writing [intra | pre] contiguously in PSUM (pre region shared per u-pair,
    accumulated in PSUM).
  - x is loaded with partition = 256-element time chunk (1KB contiguous DMA
    runs), shuffled on-chip to (u, c, v') column order (GPSIMD), transposed
    128x128 on the TensorEngine, rounded to fp32r during the batched
    PSUM->SBUF copies (ScalarE).
  - prefix totals summed across the 3 pair-regions + bias (DVE), then one
    strided combine per window writes the (o, t)-ordered output tile, stored
    with 1KB contiguous runs.
"""
import numpy as np

import concourse.bass as bass
import concourse.tile as tile
from concourse import bacc, mybir
from concourse.bass_utils import run_bass_kernel_spmd

F32 = mybir.dt.float32
F32R = mybir.dt.float32r

# problem geometry (asserted against inputs at runtime)
B, C, T, O = 8, 16, 131072, 16
P = 128
CH = 256                 # time-elements per partition per supertile
NST = T // (P * CH)      # 4 supertiles
V = 8                    # sub-block length
NU = 8                   # sub-blocks per window

_cache = {}


def _build_nc(du_count, mm_dtype=F32R):
    """Build the per-core Bass program. du_count = number of active sub-blocks
    (those with any nonzero weight), assumed to be the trailing ones."""
    S = NU * V  # 64
    NW = CH // S  # windows per partition = 4
    DU = du_count
    first_u = NU - DU          # first active sub-block
    fill_s = first_u * V       # s < fill_s -> output = bias

    nc = bacc.Bacc("TRN2", target_bir_lowering=False, debug=False)
    x_d = nc.dram_tensor("x", (C, T), F32, kind="ExternalInput")
    w_d = nc.dram_tensor("w_all", (P, DU * 256), mm_dtype, kind="ExternalInput")
    bpre_d = nc.dram_tensor("bias_pre", (P, P), F32, kind="ExternalInput")
    ident_d = nc.dram_tensor("ident", (P, P), F32, kind="ExternalInput")
    bfill_d = nc.dram_tensor("bias_fill", (P, O * fill_s), F32,
                             kind="ExternalInput")
    y_d = nc.dram_tensor("y", (O, T), F32, kind="ExternalOutput")

    xv = x_d.ap().rearrange("c (st p hs) -> st p c hs", st=NST, p=P, hs=CH)
    yv = y_d.ap().rearrange("o (st p hs) -> st p o hs", st=NST, p=P, hs=CH)

    NB = (DU + 1) // 2  # psum banks per window group

    with tile.TileContext(nc) as tc:
        with (
            tc.tile_pool(name="const", bufs=1) as cp,
            tc.tile_pool(name="io", bufs=2) as io,
            tc.tile_pool(name="mid", bufs=3) as mid,
            tc.tile_pool(name="psW", bufs=2, space="PSUM") as psW,
            tc.tile_pool(name="psT", bufs=2, space="PSUM") as psT,
        ):
            w_all = cp.tile([P, DU * 256], mm_dtype, name="w_all")
            nc.sync.dma_start(w_all[:], w_d.ap())
            bias_pre = cp.tile([P, P], F32, name="bias_pre")
            nc.sync.dma_start(bias_pre[:], bpre_d.ap())
            ident = cp.tile([P, P], F32, name="ident")
            nc.sync.dma_start(ident[:], ident_d.ap())
            bfill = cp.tile([P, O * fill_s], F32, name="bfill")
            nc.sync.dma_start(bfill[:], bfill_d.ap())

            for st in range(NST):
                xin = io.tile([P, C * CH], F32, name="xin", tag="xin")
                nc.sync.dma_start(
                    xin[:].rearrange("p (c hs) -> p c hs", c=C), xv[st]
                )
                out_sb = io.tile([P, O * CH], F32, name="out_sb", tag="out")
                for wdw in range(NW):
                    # ---- shuffle to (du, c, v') ----
                    shuf = mid.tile([P, DU * 128], F32, name="shuf", tag="shuf")
                    src = xin[:].rearrange(
                        "p (c w u v) -> w p u c v", c=C, w=NW, u=NU, v=V
                    )[wdw, :, first_u:NU]
                    nc.vector.tensor_copy(
                        shuf[:].rearrange("p (u c v) -> p u c v", u=DU, c=C, v=V),
                        src,
                    )
                    # ---- transposes (PE) in groups of <=4 per psum bank ----
                    tsb = []
                    du = 0
                    gi = 0
                    while du < DU:
                        n = min(4, DU - du)
                        pt = psT.tile([P, 512], F32, name=f"pt{gi}", tag="pt")
                        for j in range(n):
                            nc.tensor.transpose(
                                pt[:, j * 128:(j + 1) * 128],
                                shuf[:, (du + j) * 128:(du + j + 1) * 128],
                                ident[:],
                                tile_position=(0, 0),
                            )
                        ts = mid.tile([P, n * 128], mm_dtype,
                                      name=f"ts{gi}", tag=f"ts{gi}")
                        nc.scalar.copy(ts[:], pt[:, 0:n * 128])
                        for j in range(n):
                            tsb.append((ts, j))
                        du += n
                        gi += 1
                    # ---- matmuls ----
                    pw = psW.tile([P, NB * 512], F32, name="pw", tag="pw")
                    for du in range(DU):
                        bk = du // 2
                        lo = bk * 512 + (du % 2) * 128
                        ts, j = tsb[du]
                        nc.tensor.matmul(
                            pw[:, lo:lo + 256],
                            ts[:, j * 128:(j + 1) * 128],
                            w_all[:, du * 256:(du + 1) * 256],
                            start=(du % 2 == 0),
                            stop=(du % 2 == 1 or du == DU - 1),
                            skip_group_check=True,
                        )
                    # ---- prefix totals: pre_s = sum of pre regions ----
                    pre_s = mid.tile([P, P], F32, name="pre_s", tag="pre_s")
                    nc.vector.tensor_add(pre_s[:], bias_pre[:], pw[:, 128:256])
                    for bk in range(1, NB):
                        nc.vector.tensor_add(
                            pre_s[:], pre_s[:],
                            pw[:, bk * 512 + 128:bk * 512 + 256],
                        )
                    # ---- combine: out[(o, s)] = intra + pre_bcast ----
                    # out col = o*CH + wdw*S + s,  s = (first_u + du)*V + v
                    out4 = out_sb[:].rearrange(
                        "p (o w u v) -> w p o u v", o=O, w=NW, u=NU, v=V
                    )[wdw, :, :, first_u:NU]
                    # in1: psum intra: col = bk*512 + (du%2)*256 + v*16 + o
                    in1 = pw[:].rearrange(
                        "p (bk half x) -> p bk half x", bk=NB, half=2
                    )[:, :, :, 0:128]
                    in1 = in1.rearrange(
                        "p bk half (v o) -> p o (bk half) v", v=V, o=O
                    )
                    # in2: pre_s col = (first_u + du)*16 + o, step0 over v
                    in2 = pre_s[:].rearrange("p (u o) -> p u o", u=NU)
                    in2 = in2[:, first_u:NU]
                    in2 = in2.transpose([0, 2, 1]).unsqueeze(3)
                    in2 = in2.broadcast_to([P, O, DU, V])
                    nc.vector.tensor_add(out4, in1, in2)
                    # ---- bias fill for s < fill_s (ACT) ----
                    outf = out_sb[:].rearrange(
                        "p (o w s) -> w p o s", o=O, w=NW
                    )[wdw, :, :, 0:fill_s]
                    nc.scalar.copy(
                        outf,
                        bfill[:].rearrange("p (o s) -> p o s", o=O),
                    )
                nc.scalar.dma_start(
                    yv[st], out_sb[:].rearrange("p (o hs) -> p o hs", o=O)
                )
    nc.compile()
    return nc


def _host_constants(weight, bias, n_discard, n_keep, mm_np=np.float32):
    S = n_discard + n_keep
    assert S == NU * V
    w = weight.reshape(O, C, n_keep).transpose(2, 1, 0)  # (n_keep, C, O)
    w_full = np.concatenate(
        [np.zeros((n_discard, C, O), np.float32), w.astype(np.float32)], axis=0
    )  # (S, C, O)
    act = [u for u in range(NU)
           if np.abs(w_full[u * V:(u + 1) * V]).max() > 0]
    # kernel assumes active blocks are trailing & contiguous
    first_u = act[0] if act else NU
    assert act == list(range(first_u, NU))
    DU = len(act)
    rhs = np.zeros((DU, P, 256), np.float32)
    vp_idx = np.arange(V)
    for idx, u in enumerate(act):
        blk = w_full[u * V:(u + 1) * V]  # (V, C, O)
        # Wtri: k=(c,vp) -> n=(v,o)
        tri = np.zeros((C, V, V, O), np.float32)
        for v in range(V):
            tri[:, vp_idx <= v, v, :] = blk.transpose(1, 0, 2)[:, vp_idx <= v]
        Wtri = tri.reshape(C * V, V * O)
        # Wpre: k=(c,vp) -> n=(ut,o)
        pre = np.zeros((C, V, NU, O), np.float32)
        for ut in range(NU):
            if ut > u:
                pre[:, :, ut, :] = blk.transpose(1, 0, 2)
        Wpre = pre.reshape(C * V, NU * O)
        if idx % 2 == 0:
            rhs[idx] = np.concatenate([Wtri, Wpre], axis=1)
        else:
            rhs[idx] = np.concatenate([Wpre, Wtri], axis=1)
    w_all = rhs.transpose(1, 0, 2).reshape(P, DU * 256).astype(mm_np)
    bias32 = bias.astype(np.float32)
    consts = {
        "w_all": np.ascontiguousarray(w_all),
        "bias_pre": np.ascontiguousarray(
            np.tile(bias32, NU)[None, :] * np.ones((P, 1), np.float32)
        ),
        "ident": np.eye(P, dtype=np.float32),
        "bias_fill": np.ascontiguousarray(
            np.tile(bias32[:, None], (1, first_u * V)).reshape(1, -1)
            * np.ones((P, 1), np.float32)
        ),
    }
    return consts, DU


def _run(inputs, trace=False):
    x = np.asarray(inputs["x"], dtype=np.float32)
    weight = np.asarray(inputs["weight"], dtype=np.float32)
    bias = np.asarray(inputs["bias"], dtype=np.float32)
    n_discard = int(inputs["n_discard"])
    n_keep = int(inputs["n_keep"])
    assert x.shape == (B, C, T) and weight.shape == (O, C * n_keep)

    consts, DU = _host_constants(weight, bias, n_discard, n_keep)
    key = ("nc", DU)
    if key not in _cache:
        _cache[key] = _build_nc(DU)
    nc = _cache[key]

    in_maps = []
    for b in range(B):
        m = dict(consts)
        m["x"] = np.ascontiguousarray(x[b])
        in_maps.append(m)
    res = run_bass_kernel_spmd(nc, in_maps, list(range(B)), trace=trace)
    y = np.stack([res.results[b]["y"] for b in range(B)], axis=0)
    return y, res


def kernel(**inputs):
    y, _ = _run(inputs, trace=False)
    return y



# revision 5
# speedup vs baseline: 1.1359x; 1.1359x over previous
"""Trainium2 Bass kernel for nn_CumulativeFlattenedLinear (segment_reduce).

Computation: per window of S=64 timesteps, per-timestep C->O linear projection
(weights zero for the first n_discard steps) followed by a causal cumsum within
the window, plus bias.

Strategy (data-parallel over batch, 1 batch element per core):
  - x is DMA-loaded with an on-the-fly f32->bf16 cast (SWDGE), partition =
    256-element time chunk, 1KB contiguous DRAM runs.
  - Per window: GpSimd gathers the active sub-blocks into (u, c, v) column
    order (bf16); per 8-step sub-block u the 128x128 block is transposed on
    the TensorEngine (bf16: 1 cyc/row), then one bf16 matmul per sub-block
    against a host-built triangular weight block computes the intra-block
    causal cumsum of the projections: pw[p, (v,o)] in PSUM.
  - The cross-sub-block prefix comes from the intra result's last row
    (v=7 == block total) via a chain of tiny 16-col DVE adds seeded with the
    bias; one strided DVE add per window combines intra + prefix into the
    (o, t)-ordered bf16 output tile; bias fill for the discarded head (ACT).
  - y is stored as bf16 (rel err ~4e-3 << 2e-2 gate) and upcast to f32 on
    host; HBM traffic drops from 16.8MB to 12MB per core.
"""
import numpy as np

import concourse.bass as bass
import concourse.tile as tile
from concourse import bacc, mybir
from concourse.bass_utils import run_bass_kernel_spmd

F32 = mybir.dt.float32
BF16 = mybir.dt.bfloat16

# problem geometry (asserted against inputs at runtime)
B, C, T, O = 8, 16, 131072, 16
P = 128
CH = 256                 # time-elements per partition per supertile
NST = T // (P * CH)      # 4 supertiles
V = 8                    # sub-block length
NU = 8                   # sub-blocks per window

_cache = {}


def _build_nc(du_count):
    """Build the per-core Bass program. du_count = number of active sub-blocks
    (those with any nonzero weight), assumed to be the trailing ones."""
    S = NU * V  # 64
    NW = CH // S  # windows per partition = 4
    DU = du_count
    first_u = NU - DU          # first active sub-block
    fill_s = first_u * V       # s < fill_s -> output = bias

    nc = bacc.Bacc("TRN2", target_bir_lowering=False, debug=False)
    x_d = nc.dram_tensor("x", (C, T), F32, kind="ExternalInput")
    w_d = nc.dram_tensor("w_all", (P, DU * 128), BF16, kind="ExternalInput")
    b16_d = nc.dram_tensor("bias16", (P, O), F32, kind="ExternalInput")
    ident_d = nc.dram_tensor("ident", (P, P), BF16, kind="ExternalInput")
    bfill_d = nc.dram_tensor("bias_fill", (P, O * fill_s), BF16,
                             kind="ExternalInput")
    y_d = nc.dram_tensor("y", (O, T), BF16, kind="ExternalOutput")

    xv = x_d.ap().rearrange("c (st p hs) -> st p c hs", st=NST, p=P, hs=CH)
    yv = y_d.ap().rearrange("o (st p hs) -> st p o hs", st=NST, p=P, hs=CH)

    with tile.TileContext(nc) as tc:
        with (
            tc.tile_pool(name="const", bufs=1) as cp,
            tc.tile_pool(name="io", bufs=3) as io,
            tc.tile_pool(name="mid", bufs=3) as mid,
            tc.tile_pool(name="psT", bufs=2, space="PSUM") as psT,
            tc.tile_pool(name="psW", bufs=3, space="PSUM") as psW,
        ):
            w_all = cp.tile([P, DU * 128], BF16, name="w_all")
            nc.sync.dma_start(w_all[:], w_d.ap())
            bias16 = cp.tile([P, O], F32, name="bias16")
            nc.sync.dma_start(bias16[:], b16_d.ap())
            ident = cp.tile([P, P], BF16, name="ident")
            nc.sync.dma_start(ident[:], ident_d.ap())
            bfill = cp.tile([P, O * fill_s], BF16, name="bfill")
            nc.sync.dma_start(bfill[:], bfill_d.ap())

            pending = None

            xins = {}

            def prefetch(st):
                xin = io.tile([P, C * CH], BF16, name="xin", tag="xin")
                nc.gpsimd.dma_start(
                    xin[:].rearrange("p (c hs) -> p c hs", c=C), xv[st]
                )
                xins[st] = xin

            # prefetch first two supertile loads before any compute
            for st in range(min(2, NST)):
                prefetch(st)

            for st in range(NST):
                if st + 2 < NST:
                    prefetch(st + 2)
                xin = xins.pop(st)
                out_sb = io.tile([P, O * CH], BF16, name="out_sb", tag="out")
                for wdw in range(NW):
                    # ---- stage A: shuffle (Pool) + transposes (PE)
                    #      + PSUM->SBUF copy (ACT)
                    shuf = mid.tile([P, DU * 128], BF16, name="shuf",
                                    tag="shuf")
                    src = xin[:].rearrange(
                        "p (c w u v) -> w p u c v", c=C, w=NW, u=NU, v=V
                    )[wdw, :, first_u:NU]
                    nc.gpsimd.tensor_copy(
                        shuf[:].rearrange("p (u c v) -> p u c v",
                                          u=DU, c=C, v=V),
                        src,
                    )
                    pt = psT.tile([P, DU * 128], BF16, name="pt", tag="pt")
                    for i in range(DU):
                        nc.tensor.transpose(
                            pt[:, i * 128:(i + 1) * 128],
                            shuf[:, i * 128:(i + 1) * 128],
                            ident[:],
                            tile_position=(0, 0),
                        )
                    ts = mid.tile([P, DU * 128], BF16, name="ts", tag="ts")
                    nc.scalar.copy(ts[:], pt[:])

                    if pending is not None:
                        pending()

                    def stage_b(st=st, wdw=wdw, ts=ts, out_sb=out_sb):
                        # ---- matmuls: intra-block triangular projections
                        pw = psW.tile([P, DU * 128], F32, name="pw", tag="pw")
                        for i in range(DU):
                            nc.tensor.matmul(
                                pw[:, i * 128:(i + 1) * 128],
                                ts[:, i * 128:(i + 1) * 128],
                                w_all[:, i * 128:(i + 1) * 128],
                                start=True,
                                stop=True,
                            )
                        # ---- prefix chain over sub-block totals (v=7 row)
                        pre = mid.tile([P, DU * O], F32, name="pre", tag="pre")
                        nc.vector.tensor_copy(pre[:, 0:O], bias16[:])
                        for i in range(1, DU):
                            nc.vector.tensor_add(
                                pre[:, i * O:(i + 1) * O],
                                pre[:, (i - 1) * O:i * O],
                                pw[:, (i - 1) * 128 + (V - 1) * O:i * 128],
                            )
                        # ---- combine: out[(o, s)] = intra + pre_bcast
                        out4 = out_sb[:].rearrange(
                            "p (o w u v) -> w p o u v", o=O, w=NW, u=NU, v=V
                        )[wdw, :, :, first_u:NU]
                        in1 = pw[:].rearrange(
                            "p (u v o) -> p o u v", u=DU, v=V, o=O
                        )
                        in2 = pre[:].rearrange("p (u o) -> p o u", u=DU)
                        in2 = in2.unsqueeze(3).broadcast_to([P, O, DU, V])
                        nc.vector.tensor_add(out4, in1, in2)
                        # ---- bias fill for s < fill_s (ACT)
                        outf = out_sb[:].rearrange(
                            "p (o w s) -> w p o s", o=O, w=NW
                        )[wdw, :, :, 0:fill_s]
                        nc.scalar.copy(
                            outf,
                            bfill[:].rearrange("p (o s) -> p o s", o=O),
                        )
                        if wdw == NW - 1:
                            nc.scalar.dma_start(
                                yv[st],
                                out_sb[:].rearrange("p (o hs) -> p o hs", o=O),
                            )

                    pending = stage_b
            pending()
    nc.compile()
    return nc


def _host_constants(weight, bias, n_discard, n_keep):
    S = n_discard + n_keep
    assert S == NU * V
    w = weight.reshape(O, C, n_keep).transpose(2, 1, 0)  # (n_keep, C, O)
    w_full = np.concatenate(
        [np.zeros((n_discard, C, O), np.float32), w.astype(np.float32)], axis=0
    )  # (S, C, O)
    act = [u for u in range(NU)
           if np.abs(w_full[u * V:(u + 1) * V]).max() > 0]
    # kernel assumes active blocks are trailing & contiguous
    first_u = act[0] if act else NU
    assert act == list(range(first_u, NU))
    DU = len(act)
    fill_s = first_u * V
    bf16 = mybir.dt.np(BF16)
    w_all = np.zeros((P, DU * 128), np.float32)
    for idx, u in enumerate(act):
        blk = w_full[u * V:(u + 1) * V]  # (V, C, O)
        # Wtri: k=(c,vp) -> n=(v,o), vp <= v
        tri = np.zeros((C, V, V, O), np.float32)
        vp_idx = np.arange(V)
        for v in range(V):
            tri[:, vp_idx <= v, v, :] = blk.transpose(1, 0, 2)[:, vp_idx <= v]
        w_all[:, idx * 128:(idx + 1) * 128] = tri.reshape(C * V, V * O)
    bias32 = bias.astype(np.float32)
    consts = {
        "w_all": np.ascontiguousarray(w_all).astype(bf16),
        "bias16": np.ascontiguousarray(
            bias32[None, :] * np.ones((P, 1), np.float32)
        ),
        "ident": np.eye(P, dtype=np.float32).astype(bf16),
        "bias_fill": np.ascontiguousarray(
            np.tile(bias32[:, None], (1, fill_s)).reshape(1, -1)
            * np.ones((P, 1), np.float32)
        ).astype(bf16),
    }
    return consts, DU


def _run(inputs, trace=False):
    x = np.asarray(inputs["x"], dtype=np.float32)
    weight = np.asarray(inputs["weight"], dtype=np.float32)
    bias = np.asarray(inputs["bias"], dtype=np.float32)
    n_discard = int(inputs["n_discard"])
    n_keep = int(inputs["n_keep"])
    assert x.shape == (B, C, T) and weight.shape == (O, C * n_keep)

    consts, DU = _host_constants(weight, bias, n_discard, n_keep)
    key = ("nc", DU)
    if key not in _cache:
        _cache[key] = _build_nc(DU)
    nc = _cache[key]

    in_maps = []
    for b in range(B):
        m = dict(consts)
        m["x"] = np.ascontiguousarray(x[b])
        in_maps.append(m)
    res = run_bass_kernel_spmd(nc, in_maps, list(range(B)), trace=trace)
    y = np.stack(
        [res.results[b]["y"].astype(np.float32) for b in range(B)], axis=0
    )
    return y, res


def kernel(**inputs):
    y, _ = _run(inputs, trace=False)
    return y


# revision 10
# speedup vs baseline: 1.5027x; 1.3229x over previous
"""Trainium2 Bass kernel for nn_CumulativeFlattenedLinear (segment_reduce).

Computation: per window of S=64 timesteps, per-timestep C->O linear projection
(weights zero for the first n_discard steps) followed by a causal cumsum within
the window, plus bias.

Strategy (data-parallel over batch, 1 batch element per core):
  - x loaded f32 via HWDGE (sync queue), partition = 256-element time chunk,
    1KB contiguous DRAM runs; supertile 0 split in halves to start earlier.
  - Per window: DVE gathers the active sub-blocks into (u, c, v) column order,
    casting f32->bf16; per 8-step sub-block u the 128x128 block is transposed
    on the TensorEngine (bf16: 1 cyc/row), then one bf16 matmul per sub-block
    against a host-built triangular weight block (columns ordered (o, v))
    computes the intra-block causal cumsum of projections: pw[p, (o,v)] PSUM.
  - Cross-sub-block prefix: ACT copies the five block totals (v=7 lanes) to
    SBUF, GpSimd chains them with the bias seed; one strided DVE add per
    window combines intra + prefix into the (o, t)-ordered bf16 output tile;
    ACT fills the discarded head with bias.
  - y is stored as bf16 (rel err ~4e-3 << 2e-2 gate) and upcast to f32 on
    host; HBM traffic is 12MB/core (8 in + 4 out) ~= 34us at 358 GB/s.
"""
import numpy as np

import concourse.bass as bass
import concourse.tile as tile
from concourse import bacc, mybir
from concourse.bass_utils import run_bass_kernel_spmd

F32 = mybir.dt.float32
BF16 = mybir.dt.bfloat16

# problem geometry (asserted against inputs at runtime)
B, C, T, O = 8, 16, 131072, 16
P = 128
CH = 256                 # time-elements per partition per supertile
NST = T // (P * CH)      # 4 supertiles
V = 8                    # sub-block length
NU = 8                   # sub-blocks per window

_cache = {}


def _build_nc(du_count):
    """Build the per-core Bass program. du_count = number of active sub-blocks
    (those with any nonzero weight), assumed to be the trailing ones."""
    S = NU * V  # 64
    NW = CH // S  # windows per partition = 4
    DU = du_count
    first_u = NU - DU          # first active sub-block
    fill_s = first_u * V       # s < fill_s -> output = bias

    nc = bacc.Bacc("TRN2", target_bir_lowering=False, debug=False)
    x_d = nc.dram_tensor("x", (C, T), F32, kind="ExternalInput")
    w_d = nc.dram_tensor("w_all", (P, DU * 128), BF16, kind="ExternalInput")
    b16_d = nc.dram_tensor("bias16", (P, O), F32, kind="ExternalInput")
    ident_d = nc.dram_tensor("ident", (P, P), BF16, kind="ExternalInput")
    bfill_d = nc.dram_tensor("bias_fill", (P, O * fill_s), BF16,
                             kind="ExternalInput")
    y_d = nc.dram_tensor("y", (O, T), BF16, kind="ExternalOutput")

    xv = x_d.ap().rearrange("c (st p hs) -> st p c hs", st=NST, p=P, hs=CH)
    yv = y_d.ap().rearrange("o (st p hs) -> st p o hs", st=NST, p=P, hs=CH)

    with tile.TileContext(nc) as tc:
        with (
            tc.tile_pool(name="const", bufs=1) as cp,
            tc.tile_pool(name="io", bufs=3) as io,
            tc.tile_pool(name="mid", bufs=3) as mid,
            tc.tile_pool(name="psT", bufs=2, space="PSUM") as psT,
            tc.tile_pool(name="psW", bufs=3, space="PSUM") as psW,
        ):
            w_all = cp.tile([P, DU * 128], BF16, name="w_all")
            nc.sync.dma_start(w_all[:], w_d.ap())
            bias16 = cp.tile([P, O], F32, name="bias16")
            nc.sync.dma_start(bias16[:], b16_d.ap())
            ident = cp.tile([P, P], BF16, name="ident")
            nc.sync.dma_start(ident[:], ident_d.ap())
            bfill = cp.tile([P, O * fill_s], BF16, name="bfill")
            nc.sync.dma_start(bfill[:], bfill_d.ap())
            # statically-seeded prefix tiles (col 0:O = bias, never rewritten)
            pre_tiles = []
            for k in range(2):
                pre = cp.tile([P, DU * O], F32, name=f"pre{k}")
                nc.vector.tensor_copy(pre[:, 0:O], bias16[:])
                pre_tiles.append(pre)

            xins = {}

            def prefetch(st, split=False):
                xin = io.tile([P, C * CH], F32, name="xin", tag="xin")
                xr = xin[:].rearrange("p (c hs) -> p c hs", c=C)
                if split:
                    h = CH // 2
                    nc.sync.dma_start(xr[:, :, 0:h], xv[st][:, :, 0:h])
                    nc.sync.dma_start(xr[:, :, h:CH], xv[st][:, :, h:CH])
                else:
                    nc.sync.dma_start(xr, xv[st])
                xins[st] = xin

            prefetch(0, split=True)
            if NST > 1:
                prefetch(1)

            pending = None

            for st in range(NST):
                if st + 2 < NST:
                    prefetch(st + 2)
                xin = xins.pop(st)
                out_sb = io.tile([P, O * CH], BF16, name="out_sb", tag="out")
                for wdw in range(NW):
                    # ---- stage A: shuffle (DVE, f32->bf16 cast)
                    #      + transposes (PE) + PSUM->SBUF copy (ACT)
                    shuf = mid.tile([P, DU * 128], BF16, name="shuf",
                                    tag="shuf")
                    src = xin[:].rearrange(
                        "p (c w u v) -> w p u c v", c=C, w=NW, u=NU, v=V
                    )[wdw, :, first_u:NU]
                    nc.vector.tensor_copy(
                        shuf[:].rearrange("p (u c v) -> p u c v",
                                          u=DU, c=C, v=V),
                        src,
                    )
                    pt = psT.tile([P, DU * 128], BF16, name="pt", tag="pt")
                    for i in range(DU):
                        nc.tensor.transpose(
                            pt[:, i * 128:(i + 1) * 128],
                            shuf[:, i * 128:(i + 1) * 128],
                            ident[:],
                            tile_position=(0, 0),
                        )
                    ts = mid.tile([P, DU * 128], BF16, name="ts", tag="ts")
                    nc.scalar.copy(ts[:], pt[:])

                    if pending is not None:
                        pending()

                    def stage_b(st=st, wdw=wdw, ts=ts, out_sb=out_sb):
                        # ---- matmuls: intra-block triangular projections
                        #      pw columns ordered (u, o, v)
                        pw = psW.tile([P, DU * 128], F32, name="pw", tag="pw")
                        for i in range(DU):
                            nc.tensor.matmul(
                                pw[:, i * 128:(i + 1) * 128],
                                ts[:, i * 128:(i + 1) * 128],
                                w_all[:, i * 128:(i + 1) * 128],
                                start=True,
                                stop=True,
                            )
                        # ---- block totals (v = V-1 lanes) -> SBUF (ACT)
                        tot = mid.tile([P, (DU - 1) * O], F32, name="tot",
                                       tag="tot")
                        nc.scalar.copy(
                            tot[:].rearrange("p (u o) -> p u o", u=DU - 1),
                            pw[:].rearrange(
                                "p (u o v) -> p u o v", u=DU, o=O, v=V
                            )[:, 0:DU - 1, :, V - 1],
                        )
                        # ---- prefix chain with bias seed (GpSimd)
                        pre = pre_tiles[(st * NW + wdw) % 2]
                        for i in range(1, DU):
                            nc.gpsimd.tensor_add(
                                pre[:, i * O:(i + 1) * O],
                                pre[:, (i - 1) * O:i * O],
                                tot[:, (i - 1) * O:i * O],
                            )
                        # ---- combine: out[(o, s)] = intra + pre_bcast (DVE)
                        out4 = out_sb[:].rearrange(
                            "p (o w u v) -> w p o u v", o=O, w=NW, u=NU, v=V
                        )[wdw, :, :, first_u:NU]
                        in1 = pw[:].rearrange(
                            "p (u o v) -> p o u v", u=DU, o=O, v=V
                        )
                        in2 = pre[:].rearrange("p (u o) -> p o u", u=DU)
                        in2 = in2.unsqueeze(3).broadcast_to([P, O, DU, V])
                        nc.vector.tensor_add(out4, in1, in2)
                        # ---- bias fill for s < fill_s (ACT)
                        outf = out_sb[:].rearrange(
                            "p (o w s) -> w p o s", o=O, w=NW
                        )[wdw, :, :, 0:fill_s]
                        nc.scalar.copy(
                            outf,
                            bfill[:].rearrange("p (o s) -> p o s", o=O),
                        )
                        if wdw == NW - 1:
                            nc.scalar.dma_start(
                                yv[st],
                                out_sb[:].rearrange("p (o hs) -> p o hs", o=O),
                            )

                    pending = stage_b
            pending()
    nc.compile()
    return nc


def _host_constants(weight, bias, n_discard, n_keep):
    S = n_discard + n_keep
    assert S == NU * V
    w = weight.reshape(O, C, n_keep).transpose(2, 1, 0)  # (n_keep, C, O)
    w_full = np.concatenate(
        [np.zeros((n_discard, C, O), np.float32), w.astype(np.float32)], axis=0
    )  # (S, C, O)
    act = [u for u in range(NU)
           if np.abs(w_full[u * V:(u + 1) * V]).max() > 0]
    # kernel assumes active blocks are trailing & contiguous
    first_u = act[0] if act else NU
    assert act == list(range(first_u, NU))
    DU = len(act)
    fill_s = first_u * V
    bf16 = mybir.dt.np(BF16)
    w_all = np.zeros((P, DU * 128), np.float32)
    for idx, u in enumerate(act):
        blk = w_full[u * V:(u + 1) * V]  # (V, C, O)
        # Wtri: k=(c,vp) -> n=(o,v), vp <= v
        tri = np.zeros((C, V, O, V), np.float32)
        for v in range(V):
            for vp in range(v + 1):
                tri[:, vp, :, v] = blk[vp]
        w_all[:, idx * 128:(idx + 1) * 128] = tri.reshape(C * V, O * V)
    bias32 = bias.astype(np.float32)
    consts = {
        "w_all": np.ascontiguousarray(w_all).astype(bf16),
        "bias16": np.ascontiguousarray(
            bias32[None, :] * np.ones((P, 1), np.float32)
        ),
        "ident": np.eye(P, dtype=np.float32).astype(bf16),
        "bias_fill": np.ascontiguousarray(
            np.tile(bias32[:, None], (1, fill_s)).reshape(1, -1)
            * np.ones((P, 1), np.float32)
        ).astype(bf16),
    }
    return consts, DU


def _run(inputs, trace=False):
    x = np.asarray(inputs["x"], dtype=np.float32)
    weight = np.asarray(inputs["weight"], dtype=np.float32)
    bias = np.asarray(inputs["bias"], dtype=np.float32)
    n_discard = int(inputs["n_discard"])
    n_keep = int(inputs["n_keep"])
    assert x.shape == (B, C, T) and weight.shape == (O, C * n_keep)

    consts, DU = _host_constants(weight, bias, n_discard, n_keep)
    key = ("nc", DU)
    if key not in _cache:
        _cache[key] = _build_nc(DU)
    nc = _cache[key]

    in_maps = []
    for b in range(B):
        m = dict(consts)
        m["x"] = np.ascontiguousarray(x[b])
        in_maps.append(m)
    res = run_bass_kernel_spmd(nc, in_maps, list(range(B)), trace=trace)
    y = np.stack(
        [res.results[b]["y"].astype(np.float32) for b in range(B)], axis=0
    )
    return y, res


def kernel(**inputs):
    y, _ = _run(inputs, trace=False)
    return y


# revision 14
# speedup vs baseline: 1.5228x; 1.0133x over previous
"""Trainium2 Bass kernel for nn_CumulativeFlattenedLinear (segment_reduce).

Computation: per window of S=64 timesteps, per-timestep C->O linear projection
(weights zero for the first n_discard steps) followed by a causal cumsum within
the window, plus bias.

Strategy (data-parallel over batch, 1 batch element per core):
  - x loaded f32 via HWDGE (sync queue), partition = 256-element time chunk,
    1KB contiguous DRAM runs; supertile 0 split in halves to start earlier.
  - Per window: DVE gathers the active sub-blocks into (u, c, v) column order,
    casting f32->bf16; per 8-step sub-block u the 128x128 block is transposed
    on the TensorEngine (bf16: 1 cyc/row), then one bf16 matmul per sub-block
    against a host-built triangular weight block (columns ordered (o, v))
    computes the intra-block causal cumsum of projections: pw[p, (o,v)] PSUM.
  - Cross-sub-block prefix: ACT copies the five block totals (v=7 lanes) to
    SBUF, GpSimd chains them with the bias seed; one strided DVE add per
    window combines intra + prefix into the (o, t)-ordered bf16 output tile;
    ACT fills the discarded head with bias.
  - y is stored as bf16 (rel err ~4e-3 << 2e-2 gate) and upcast to f32 on
    host; HBM traffic is 12MB/core (8 in + 4 out) ~= 34us at 358 GB/s.
"""
import numpy as np

import concourse.bass as bass
import concourse.tile as tile
from concourse import bacc, mybir
from concourse.bass_utils import run_bass_kernel_spmd

F32 = mybir.dt.float32
BF16 = mybir.dt.bfloat16

# problem geometry (asserted against inputs at runtime)
B, C, T, O = 8, 16, 131072, 16
P = 128
CH = 256                 # time-elements per partition per supertile
NST = T // (P * CH)      # 4 supertiles
V = 8                    # sub-block length
NU = 8                   # sub-blocks per window

_cache = {}


def _build_nc(du_count):
    """Build the per-core Bass program. du_count = number of active sub-blocks
    (those with any nonzero weight), assumed to be the trailing ones."""
    S = NU * V  # 64
    NW = CH // S  # windows per partition = 4
    DU = du_count
    first_u = NU - DU          # first active sub-block
    fill_s = first_u * V       # s < fill_s -> output = bias

    nc = bacc.Bacc("TRN2", target_bir_lowering=False, debug=False)
    x_d = nc.dram_tensor("x", (C, T), F32, kind="ExternalInput")
    w_d = nc.dram_tensor("w_all", (P, DU * 128), BF16, kind="ExternalInput")
    b16_d = nc.dram_tensor("bias16", (P, O), F32, kind="ExternalInput")
    ident_d = nc.dram_tensor("ident", (P, P), BF16, kind="ExternalInput")
    bfill_d = nc.dram_tensor("bias_fill", (P, O * fill_s), BF16,
                             kind="ExternalInput")
    y_d = nc.dram_tensor("y", (O, T), BF16, kind="ExternalOutput")

    xv = x_d.ap().rearrange("c (st p hs) -> st p c hs", st=NST, p=P, hs=CH)
    yv = y_d.ap().rearrange("o (st p hs) -> st p o hs", st=NST, p=P, hs=CH)

    with tile.TileContext(nc) as tc:
        SKEW = 3
        with (
            tc.tile_pool(name="const", bufs=1) as cp,
            tc.tile_pool(name="io", bufs=3) as io,
            tc.tile_pool(name="mid", bufs=SKEW + 2) as mid,
            tc.tile_pool(name="psT", bufs=2, space="PSUM") as psT,
            tc.tile_pool(name="psW", bufs=3, space="PSUM") as psW,
        ):
            xins = {}

            def prefetch(st, split=False):
                xin = io.tile([P, C * CH], F32, name="xin", tag="xin")
                xr = xin[:].rearrange("p (c hs) -> p c hs", c=C)
                if split:
                    h = CH // 2
                    nc.sync.dma_start(xr[:, :, 0:h], xv[st][:, :, 0:h])
                    nc.sync.dma_start(xr[:, :, h:CH], xv[st][:, :, h:CH])
                else:
                    nc.sync.dma_start(xr, xv[st])
                xins[st] = xin

            # first half-supertile load goes out before the consts
            xin0 = io.tile([P, C * CH], F32, name="xin", tag="xin")
            xr0 = xin0[:].rearrange("p (c hs) -> p c hs", c=C)
            h = CH // 2
            nc.sync.dma_start(xr0[:, :, 0:h], xv[0][:, :, 0:h])
            xins[0] = xin0

            w_all = cp.tile([P, DU * 128], BF16, name="w_all")
            nc.sync.dma_start(w_all[:], w_d.ap())
            bias16 = cp.tile([P, O], F32, name="bias16")
            nc.sync.dma_start(bias16[:], b16_d.ap())
            ident = cp.tile([P, P], BF16, name="ident")
            nc.sync.dma_start(ident[:], ident_d.ap())
            bfill = cp.tile([P, O * fill_s], BF16, name="bfill")
            nc.sync.dma_start(bfill[:], bfill_d.ap())
            # statically-seeded prefix tiles (col 0:O = bias, never rewritten)
            pre_tiles = []
            for k in range(4):
                pre = cp.tile([P, DU * O], F32, name=f"pre{k}")
                nc.vector.tensor_copy(pre[:, 0:O], bias16[:])
                pre_tiles.append(pre)

            nc.sync.dma_start(xr0[:, :, h:CH], xv[0][:, :, h:CH])
            if NST > 1:
                prefetch(1)

            pending = []

            for st in range(NST):
                if st + 2 < NST:
                    prefetch(st + 2)
                xin = xins.pop(st)
                out_sb = io.tile([P, O * CH], BF16, name="out_sb", tag="out")
                for wdw in range(NW):
                    # ---- stage A: shuffle (DVE, f32->bf16 cast)
                    #      + transposes (PE) + PSUM->SBUF copy (ACT)
                    shuf = mid.tile([P, DU * 128], BF16, name="shuf",
                                    tag="shuf")
                    src = xin[:].rearrange(
                        "p (c w u v) -> w p u c v", c=C, w=NW, u=NU, v=V
                    )[wdw, :, first_u:NU]
                    nc.vector.tensor_copy(
                        shuf[:].rearrange("p (u c v) -> p u c v",
                                          u=DU, c=C, v=V),
                        src,
                    )
                    pt = psT.tile([P, DU * 128], BF16, name="pt", tag="pt")
                    for i in range(DU):
                        nc.tensor.transpose(
                            pt[:, i * 128:(i + 1) * 128],
                            shuf[:, i * 128:(i + 1) * 128],
                            ident[:],
                            tile_position=(0, 0),
                        )
                    ts = mid.tile([P, DU * 128], BF16, name="ts", tag="ts")
                    nc.scalar.copy(ts[:], pt[:])

                    if len(pending) >= SKEW:
                        pending.pop(0)()

                    def stage_b(st=st, wdw=wdw, ts=ts, out_sb=out_sb):
                        # ---- matmuls: intra-block triangular projections
                        #      pw columns ordered (u, o, v)
                        pw = psW.tile([P, DU * 128], F32, name="pw", tag="pw")
                        for i in range(DU):
                            nc.tensor.matmul(
                                pw[:, i * 128:(i + 1) * 128],
                                ts[:, i * 128:(i + 1) * 128],
                                w_all[:, i * 128:(i + 1) * 128],
                                start=True,
                                stop=True,
                            )
                        # ---- block totals (v = V-1 lanes) -> SBUF (ACT)
                        tot = mid.tile([P, (DU - 1) * O], F32, name="tot",
                                       tag="tot")
                        nc.scalar.copy(
                            tot[:].rearrange("p (u o) -> p u o", u=DU - 1),
                            pw[:].rearrange(
                                "p (u o v) -> p u o v", u=DU, o=O, v=V
                            )[:, 0:DU - 1, :, V - 1],
                        )
                        # ---- prefix chain with bias seed (GpSimd)
                        pre = pre_tiles[(st * NW + wdw) % 4]
                        for i in range(1, DU):
                            nc.gpsimd.tensor_add(
                                pre[:, i * O:(i + 1) * O],
                                pre[:, (i - 1) * O:i * O],
                                tot[:, (i - 1) * O:i * O],
                            )
                        # ---- combine: out[(o, s)] = intra + pre_bcast (DVE)
                        out4 = out_sb[:].rearrange(
                            "p (o w u v) -> w p o u v", o=O, w=NW, u=NU, v=V
                        )[wdw, :, :, first_u:NU]
                        in1 = pw[:].rearrange(
                            "p (u o v) -> p o u v", u=DU, o=O, v=V
                        )
                        in2 = pre[:].rearrange("p (u o) -> p o u", u=DU)
                        in2 = in2.unsqueeze(3).broadcast_to([P, O, DU, V])
                        nc.vector.tensor_add(out4, in1, in2)
                        if wdw == 0:
                            # ---- bias fill for s < fill_s, all windows (ACT)
                            outf = out_sb[:].rearrange(
                                "p (o w s) -> p o w s", o=O, w=NW
                            )[:, :, :, 0:fill_s]
                            nc.scalar.copy(
                                outf,
                                bfill[:].rearrange("p (o s) -> p o s", o=O)
                                .unsqueeze(2).broadcast_to([P, O, NW, fill_s]),
                            )
                        if wdw == NW - 1:
                            nc.scalar.dma_start(
                                yv[st],
                                out_sb[:].rearrange("p (o hs) -> p o hs", o=O),
                            )

                    pending.append(stage_b)
            for fn in pending:
                fn()
    nc.compile()
    return nc


def _host_constants(weight, bias, n_discard, n_keep):
    S = n_discard + n_keep
    assert S == NU * V
    w = weight.reshape(O, C, n_keep).transpose(2, 1, 0)  # (n_keep, C, O)
    w_full = np.concatenate(
        [np.zeros((n_discard, C, O), np.float32), w.astype(np.float32)], axis=0
    )  # (S, C, O)
    act = [u for u in range(NU)
           if np.abs(w_full[u * V:(u + 1) * V]).max() > 0]
    # kernel assumes active blocks are trailing & contiguous
    first_u = act[0] if act else NU
    assert act == list(range(first_u, NU))
    DU = len(act)
    fill_s = first_u * V
    bf16 = mybir.dt.np(BF16)
    w_all = np.zeros((P, DU * 128), np.float32)
    for idx, u in enumerate(act):
        blk = w_full[u * V:(u + 1) * V]  # (V, C, O)
        # Wtri: k=(c,vp) -> n=(o,v), vp <= v
        tri = np.zeros((C, V, O, V), np.float32)
        for v in range(V):
            for vp in range(v + 1):
                tri[:, vp, :, v] = blk[vp]
        w_all[:, idx * 128:(idx + 1) * 128] = tri.reshape(C * V, O * V)
    bias32 = bias.astype(np.float32)
    consts = {
        "w_all": np.ascontiguousarray(w_all).astype(bf16),
        "bias16": np.ascontiguousarray(
            bias32[None, :] * np.ones((P, 1), np.float32)
        ),
        "ident": np.eye(P, dtype=np.float32).astype(bf16),
        "bias_fill": np.ascontiguousarray(
            np.tile(bias32[:, None], (1, fill_s)).reshape(1, -1)
            * np.ones((P, 1), np.float32)
        ).astype(bf16),
    }
    return consts, DU


def _run(inputs, trace=False):
    x = np.asarray(inputs["x"], dtype=np.float32)
    weight = np.asarray(inputs["weight"], dtype=np.float32)
    bias = np.asarray(inputs["bias"], dtype=np.float32)
    n_discard = int(inputs["n_discard"])
    n_keep = int(inputs["n_keep"])
    assert x.shape == (B, C, T) and weight.shape == (O, C * n_keep)

    consts, DU = _host_constants(weight, bias, n_discard, n_keep)
    key = ("nc", DU)
    if key not in _cache:
        _cache[key] = _build_nc(DU)
    nc = _cache[key]

    in_maps = []
    for b in range(B):
        m = dict(consts)
        m["x"] = np.ascontiguousarray(x[b])
        in_maps.append(m)
    res = run_bass_kernel_spmd(nc, in_maps, list(range(B)), trace=trace)
    y = np.stack(
        [res.results[b]["y"].astype(np.float32) for b in range(B)], axis=0
    )
    return y, res


def kernel(**inputs):
    y, _ = _run(inputs, trace=False)
    return y


# revision 15
# speedup vs baseline: 1.5249x; 1.0014x over previous
"""Trainium2 Bass kernel for nn_CumulativeFlattenedLinear (segment_reduce).

Computation: per window of S=64 timesteps, per-timestep C->O linear projection
(weights zero for the first n_discard steps) followed by a causal cumsum within
the window, plus bias.

Strategy (data-parallel over batch, 1 batch element per core):
  - x loaded f32 via HWDGE (sync queue), partition = 256-element time chunk,
    1KB contiguous DRAM runs; supertile 0 split in halves to start earlier.
  - Per window: DVE gathers the active sub-blocks into (u, c, v) column order,
    casting f32->bf16; per 8-step sub-block u the 128x128 block is transposed
    on the TensorEngine (bf16: 1 cyc/row), then one bf16 matmul per sub-block
    against a host-built triangular weight block (columns ordered (o, v))
    computes the intra-block causal cumsum of projections: pw[p, (o,v)] PSUM.
  - Cross-sub-block prefix: ACT copies the five block totals (v=7 lanes) to
    SBUF, GpSimd chains them with the bias seed; one strided DVE add per
    window combines intra + prefix into the (o, t)-ordered bf16 output tile;
    ACT fills the discarded head with bias.
  - y is stored as bf16 (rel err ~4e-3 << 2e-2 gate) and upcast to f32 on
    host; HBM traffic is 12MB/core (8 in + 4 out) ~= 34us at 358 GB/s.
"""
import numpy as np

import concourse.bass as bass
import concourse.tile as tile
from concourse import bacc, mybir
from concourse.bass_utils import run_bass_kernel_spmd

F32 = mybir.dt.float32
BF16 = mybir.dt.bfloat16

# problem geometry (asserted against inputs at runtime)
B, C, T, O = 8, 16, 131072, 16
P = 128
CH = 256                 # time-elements per partition per supertile
NST = T // (P * CH)      # 4 supertiles
V = 8                    # sub-block length
NU = 8                   # sub-blocks per window

_cache = {}


def _build_nc(du_count):
    """Build the per-core Bass program. du_count = number of active sub-blocks
    (those with any nonzero weight), assumed to be the trailing ones."""
    S = NU * V  # 64
    NW = CH // S  # windows per partition = 4
    DU = du_count
    first_u = NU - DU          # first active sub-block
    fill_s = first_u * V       # s < fill_s -> output = bias

    nc = bacc.Bacc("TRN2", target_bir_lowering=False, debug=False)
    x_d = nc.dram_tensor("x", (C, T), F32, kind="ExternalInput")
    w_d = nc.dram_tensor("w_all", (P, DU * 128), BF16, kind="ExternalInput")
    b16_d = nc.dram_tensor("bias16", (P, O), F32, kind="ExternalInput")
    ident_d = nc.dram_tensor("ident", (P, P), BF16, kind="ExternalInput")
    bfill_d = nc.dram_tensor("bias_fill", (P, O * fill_s), BF16,
                             kind="ExternalInput")
    y_d = nc.dram_tensor("y", (O, T), BF16, kind="ExternalOutput")

    xv = x_d.ap().rearrange("c (st p hs) -> st p c hs", st=NST, p=P, hs=CH)
    yv = y_d.ap().rearrange("o (st p hs) -> st p o hs", st=NST, p=P, hs=CH)

    with tile.TileContext(nc) as tc:
        SKEW = 3
        with (
            tc.tile_pool(name="const", bufs=1) as cp,
            tc.tile_pool(name="io", bufs=3) as io,
            tc.tile_pool(name="mid", bufs=SKEW + 2) as mid,
            tc.tile_pool(name="psT", bufs=2, space="PSUM") as psT,
            tc.tile_pool(name="psW", bufs=3, space="PSUM") as psW,
        ):
            xins = {}

            def prefetch(st, split=False):
                xin = io.tile([P, C * CH], F32, name="xin", tag="xin")
                xr = xin[:].rearrange("p (c hs) -> p c hs", c=C)
                if split:
                    h = CH // 2
                    nc.sync.dma_start(xr[:, :, 0:h], xv[st][:, :, 0:h])
                    nc.sync.dma_start(xr[:, :, h:CH], xv[st][:, :, h:CH])
                else:
                    nc.sync.dma_start(xr, xv[st])
                xins[st] = xin

            # first half-supertile load goes out before the consts
            xin0 = io.tile([P, C * CH], F32, name="xin", tag="xin")
            xr0 = xin0[:].rearrange("p (c hs) -> p c hs", c=C)
            h = CH // 2
            nc.sync.dma_start(xr0[:, :, 0:h], xv[0][:, :, 0:h])
            xins[0] = xin0

            w_all = cp.tile([P, DU * 128], BF16, name="w_all")
            nc.scalar.dma_start(w_all[:], w_d.ap())
            bias16 = cp.tile([P, O], F32, name="bias16")
            nc.scalar.dma_start(bias16[:], b16_d.ap())
            ident = cp.tile([P, P], BF16, name="ident")
            nc.scalar.dma_start(ident[:], ident_d.ap())
            bfill = cp.tile([P, O * fill_s], BF16, name="bfill")
            nc.scalar.dma_start(bfill[:], bfill_d.ap())
            # statically-seeded prefix tiles (col 0:O = bias, never rewritten)
            pre_tiles = []
            for k in range(4):
                pre = cp.tile([P, DU * O], F32, name=f"pre{k}")
                nc.vector.tensor_copy(pre[:, 0:O], bias16[:])
                pre_tiles.append(pre)

            nc.sync.dma_start(xr0[:, :, h:CH], xv[0][:, :, h:CH])
            if NST > 1:
                prefetch(1)

            pending = []

            for st in range(NST):
                if st + 2 < NST:
                    prefetch(st + 2)
                xin = xins.pop(st)
                out_sb = io.tile([P, O * CH], BF16, name="out_sb", tag="out")
                for wdw in range(NW):
                    # ---- stage A: shuffle (DVE, f32->bf16 cast)
                    #      + transposes (PE) + PSUM->SBUF copy (ACT)
                    shuf = mid.tile([P, DU * 128], BF16, name="shuf",
                                    tag="shuf")
                    src = xin[:].rearrange(
                        "p (c w u v) -> w p u c v", c=C, w=NW, u=NU, v=V
                    )[wdw, :, first_u:NU]
                    nc.vector.tensor_copy(
                        shuf[:].rearrange("p (u c v) -> p u c v",
                                          u=DU, c=C, v=V),
                        src,
                    )
                    pt = psT.tile([P, DU * 128], BF16, name="pt", tag="pt")
                    for i in range(DU):
                        nc.tensor.transpose(
                            pt[:, i * 128:(i + 1) * 128],
                            shuf[:, i * 128:(i + 1) * 128],
                            ident[:],
                            tile_position=(0, 0),
                        )
                    ts = mid.tile([P, DU * 128], BF16, name="ts", tag="ts")
                    nc.scalar.copy(ts[:], pt[:])

                    if len(pending) >= SKEW:
                        pending.pop(0)()

                    def stage_b(st=st, wdw=wdw, ts=ts, out_sb=out_sb):
                        # ---- matmuls: intra-block triangular projections
                        #      pw columns ordered (u, o, v)
                        pw = psW.tile([P, DU * 128], F32, name="pw", tag="pw")
                        for i in range(DU):
                            nc.tensor.matmul(
                                pw[:, i * 128:(i + 1) * 128],
                                ts[:, i * 128:(i + 1) * 128],
                                w_all[:, i * 128:(i + 1) * 128],
                                start=True,
                                stop=True,
                            )
                        # ---- block totals (v = V-1 lanes) -> SBUF (ACT)
                        tot = mid.tile([P, (DU - 1) * O], F32, name="tot",
                                       tag="tot")
                        nc.scalar.copy(
                            tot[:].rearrange("p (u o) -> p u o", u=DU - 1),
                            pw[:].rearrange(
                                "p (u o v) -> p u o v", u=DU, o=O, v=V
                            )[:, 0:DU - 1, :, V - 1],
                        )
                        # ---- prefix chain with bias seed (GpSimd)
                        pre = pre_tiles[(st * NW + wdw) % 4]
                        for i in range(1, DU):
                            nc.gpsimd.tensor_add(
                                pre[:, i * O:(i + 1) * O],
                                pre[:, (i - 1) * O:i * O],
                                tot[:, (i - 1) * O:i * O],
                            )
                        # ---- combine: out[(o, s)] = intra + pre_bcast (DVE)
                        out4 = out_sb[:].rearrange(
                            "p (o w u v) -> w p o u v", o=O, w=NW, u=NU, v=V
                        )[wdw, :, :, first_u:NU]
                        in1 = pw[:].rearrange(
                            "p (u o v) -> p o u v", u=DU, o=O, v=V
                        )
                        in2 = pre[:].rearrange("p (u o) -> p o u", u=DU)
                        in2 = in2.unsqueeze(3).broadcast_to([P, O, DU, V])
                        nc.vector.tensor_add(out4, in1, in2)
                        if wdw == 0:
                            # ---- bias fill for s < fill_s, all windows (ACT)
                            outf = out_sb[:].rearrange(
                                "p (o w s) -> p o w s", o=O, w=NW
                            )[:, :, :, 0:fill_s]
                            nc.scalar.copy(
                                outf,
                                bfill[:].rearrange("p (o s) -> p o s", o=O)
                                .unsqueeze(2).broadcast_to([P, O, NW, fill_s]),
                            )
                        if wdw == NW - 1:
                            nc.scalar.dma_start(
                                yv[st],
                                out_sb[:].rearrange("p (o hs) -> p o hs", o=O),
                            )

                    pending.append(stage_b)
            for fn in pending:
                fn()
    nc.compile()
    return nc


def _host_constants(weight, bias, n_discard, n_keep):
    S = n_discard + n_keep
    assert S == NU * V
    w = weight.reshape(O, C, n_keep).transpose(2, 1, 0)  # (n_keep, C, O)
    w_full = np.concatenate(
        [np.zeros((n_discard, C, O), np.float32), w.astype(np.float32)], axis=0
    )  # (S, C, O)
    act = [u for u in range(NU)
           if np.abs(w_full[u * V:(u + 1) * V]).max() > 0]
    # kernel assumes active blocks are trailing & contiguous
    first_u = act[0] if act else NU
    assert act == list(range(first_u, NU))
    DU = len(act)
    fill_s = first_u * V
    bf16 = mybir.dt.np(BF16)
    w_all = np.zeros((P, DU * 128), np.float32)
    for idx, u in enumerate(act):
        blk = w_full[u * V:(u + 1) * V]  # (V, C, O)
        # Wtri: k=(c,vp) -> n=(o,v), vp <= v
        tri = np.zeros((C, V, O, V), np.float32)
        for v in range(V):
            for vp in range(v + 1):
                tri[:, vp, :, v] = blk[vp]
        w_all[:, idx * 128:(idx + 1) * 128] = tri.reshape(C * V, O * V)
    bias32 = bias.astype(np.float32)
    consts = {
        "w_all": np.ascontiguousarray(w_all).astype(bf16),
        "bias16": np.ascontiguousarray(
            bias32[None, :] * np.ones((P, 1), np.float32)
        ),
        "ident": np.eye(P, dtype=np.float32).astype(bf16),
        "bias_fill": np.ascontiguousarray(
            np.tile(bias32[:, None], (1, fill_s)).reshape(1, -1)
            * np.ones((P, 1), np.float32)
        ).astype(bf16),
    }
    return consts, DU


def _run(inputs, trace=False):
    x = np.asarray(inputs["x"], dtype=np.float32)
    weight = np.asarray(inputs["weight"], dtype=np.float32)
    bias = np.asarray(inputs["bias"], dtype=np.float32)
    n_discard = int(inputs["n_discard"])
    n_keep = int(inputs["n_keep"])
    assert x.shape == (B, C, T) and weight.shape == (O, C * n_keep)

    consts, DU = _host_constants(weight, bias, n_discard, n_keep)
    key = ("nc", DU)
    if key not in _cache:
        _cache[key] = _build_nc(DU)
    nc = _cache[key]

    in_maps = []
    for b in range(B):
        m = dict(consts)
        m["x"] = np.ascontiguousarray(x[b])
        in_maps.append(m)
    res = run_bass_kernel_spmd(nc, in_maps, list(range(B)), trace=trace)
    y = np.stack(
        [res.results[b]["y"].astype(np.float32) for b in range(B)], axis=0
    )
    return y, res


def kernel(**inputs):
    y, _ = _run(inputs, trace=False)
    return y


# revision 17
# speedup vs baseline: 1.5277x; 1.0019x over previous
"""Trainium2 Bass kernel for nn_CumulativeFlattenedLinear (segment_reduce).

Computation: per window of S=64 timesteps, per-timestep C->O linear projection
(weights zero for the first n_discard steps) followed by a causal cumsum within
the window, plus bias.

Strategy (data-parallel over batch, 1 batch element per core):
  - x loaded f32 via HWDGE (sync queue), partition = 256-element time chunk,
    1KB contiguous DRAM runs; supertile 0 split in halves to start earlier.
  - Per window: DVE gathers the active sub-blocks into (u, c, v) column order,
    casting f32->bf16; per 8-step sub-block u the 128x128 block is transposed
    on the TensorEngine (bf16: 1 cyc/row), then one bf16 matmul per sub-block
    against a host-built triangular weight block (columns ordered (o, v))
    computes the intra-block causal cumsum of projections: pw[p, (o,v)] PSUM.
  - Cross-sub-block prefix: ACT copies the five block totals (v=7 lanes) to
    SBUF, GpSimd chains them with the bias seed; one strided DVE add per
    window combines intra + prefix into the (o, t)-ordered bf16 output tile;
    ACT fills the discarded head with bias.
  - y is stored as bf16 (rel err ~4e-3 << 2e-2 gate) and upcast to f32 on
    host; HBM traffic is 12MB/core (8 in + 4 out) ~= 34us at 358 GB/s.
"""
import numpy as np

import concourse.bass as bass
import concourse.tile as tile
from concourse import bacc, mybir
from concourse.bass_utils import run_bass_kernel_spmd

F32 = mybir.dt.float32
BF16 = mybir.dt.bfloat16

# problem geometry (asserted against inputs at runtime)
B, C, T, O = 8, 16, 131072, 16
P = 128
CH = 256                 # time-elements per partition per supertile
NST = T // (P * CH)      # 4 supertiles
V = 8                    # sub-block length
NU = 8                   # sub-blocks per window

_cache = {}


def _build_nc(du_count):
    """Build the per-core Bass program. du_count = number of active sub-blocks
    (those with any nonzero weight), assumed to be the trailing ones."""
    S = NU * V  # 64
    NW = CH // S  # windows per partition = 4
    DU = du_count
    first_u = NU - DU          # first active sub-block
    fill_s = first_u * V       # s < fill_s -> output = bias

    nc = bacc.Bacc("TRN2", target_bir_lowering=False, debug=False)
    x_d = nc.dram_tensor("x", (C, T), F32, kind="ExternalInput")
    w_d = nc.dram_tensor("w_all", (P, DU * 128), BF16, kind="ExternalInput")
    b16_d = nc.dram_tensor("bias16", (P, O), F32, kind="ExternalInput")
    ident_d = nc.dram_tensor("ident", (P, P), BF16, kind="ExternalInput")
    bfill_d = nc.dram_tensor("bias_fill", (P, O * fill_s), BF16,
                             kind="ExternalInput")
    y_d = nc.dram_tensor("y", (O, T), BF16, kind="ExternalOutput")

    xv = x_d.ap().rearrange("c (st p hs) -> st p c hs", st=NST, p=P, hs=CH)
    yv = y_d.ap().rearrange("o (st p hs) -> st p o hs", st=NST, p=P, hs=CH)

    with tile.TileContext(nc) as tc:
        SKEW = 3
        with (
            tc.tile_pool(name="const", bufs=1) as cp,
            tc.tile_pool(name="io", bufs=3) as io,
            tc.tile_pool(name="mid", bufs=SKEW + 2) as mid,
            tc.tile_pool(name="psT", bufs=2, space="PSUM") as psT,
            tc.tile_pool(name="psW", bufs=3, space="PSUM") as psW,
        ):
            xins = {}
            h = CH // 2

            def prefetch(st, second_half_only=False):
                if not second_half_only:
                    xin = io.tile([P, C * CH], F32, name="xin", tag="xin")
                    xins[st] = xin
                xr = xins[st][:].rearrange("p (c hs) -> p c hs", c=C)
                if second_half_only:
                    nc.sync.dma_start(xr[:, :, h:CH], xv[st][:, :, h:CH])
                else:
                    nc.sync.dma_start(xr[:, :, 0:h], xv[st][:, :, 0:h])

            # first half-supertile load goes out before the consts
            prefetch(0)

            w_all = cp.tile([P, DU * 128], BF16, name="w_all")
            nc.scalar.dma_start(w_all[:], w_d.ap())
            bias16 = cp.tile([P, O], F32, name="bias16")
            nc.scalar.dma_start(bias16[:], b16_d.ap())
            ident = cp.tile([P, P], BF16, name="ident")
            nc.scalar.dma_start(ident[:], ident_d.ap())
            bfill = cp.tile([P, O * fill_s], BF16, name="bfill")
            nc.scalar.dma_start(bfill[:], bfill_d.ap())
            # statically-seeded prefix tiles (col 0:O = bias, never rewritten)
            pre_tiles = []
            for k in range(4):
                pre = cp.tile([P, DU * O], F32, name=f"pre{k}")
                nc.vector.tensor_copy(pre[:, 0:O], bias16[:])
                pre_tiles.append(pre)

            prefetch(0, second_half_only=True)
            if NST > 1:
                prefetch(1)
                prefetch(1, second_half_only=True)

            pending = []

            for st in range(NST):
                if st + 2 < NST:
                    prefetch(st + 2)
                    prefetch(st + 2, second_half_only=True)
                xin = xins.pop(st)
                out_sb = io.tile([P, O * CH], BF16, name="out_sb", tag="out")
                for wdw in range(NW):
                    # ---- stage A: shuffle (DVE, f32->bf16 cast)
                    #      + transposes (PE) + PSUM->SBUF copy (ACT)
                    shuf = mid.tile([P, DU * 128], BF16, name="shuf",
                                    tag="shuf")
                    src = xin[:].rearrange(
                        "p (c w u v) -> w p u c v", c=C, w=NW, u=NU, v=V
                    )[wdw, :, first_u:NU]
                    nc.vector.tensor_copy(
                        shuf[:].rearrange("p (u c v) -> p u c v",
                                          u=DU, c=C, v=V),
                        src,
                    )
                    pt = psT.tile([P, DU * 128], BF16, name="pt", tag="pt")
                    for i in range(DU):
                        nc.tensor.transpose(
                            pt[:, i * 128:(i + 1) * 128],
                            shuf[:, i * 128:(i + 1) * 128],
                            ident[:],
                            tile_position=(0, 0),
                        )
                    ts = mid.tile([P, DU * 128], BF16, name="ts", tag="ts")
                    nc.scalar.copy(ts[:], pt[:])

                    if len(pending) >= SKEW:
                        pending.pop(0)()

                    def stage_b(st=st, wdw=wdw, ts=ts, out_sb=out_sb):
                        # ---- matmuls: intra-block triangular projections
                        #      pw columns ordered (u, o, v)
                        pw = psW.tile([P, DU * 128], F32, name="pw", tag="pw")
                        for i in range(DU):
                            nc.tensor.matmul(
                                pw[:, i * 128:(i + 1) * 128],
                                ts[:, i * 128:(i + 1) * 128],
                                w_all[:, i * 128:(i + 1) * 128],
                                start=True,
                                stop=True,
                            )
                        # ---- block totals (v = V-1 lanes) -> SBUF (ACT)
                        tot = mid.tile([P, (DU - 1) * O], F32, name="tot",
                                       tag="tot")
                        nc.scalar.copy(
                            tot[:].rearrange("p (u o) -> p u o", u=DU - 1),
                            pw[:].rearrange(
                                "p (u o v) -> p u o v", u=DU, o=O, v=V
                            )[:, 0:DU - 1, :, V - 1],
                        )
                        # ---- prefix chain with bias seed (GpSimd)
                        pre = pre_tiles[(st * NW + wdw) % 4]
                        for i in range(1, DU):
                            nc.gpsimd.tensor_add(
                                pre[:, i * O:(i + 1) * O],
                                pre[:, (i - 1) * O:i * O],
                                tot[:, (i - 1) * O:i * O],
                            )
                        # ---- combine: out[(o, s)] = intra + pre_bcast (DVE)
                        out4 = out_sb[:].rearrange(
                            "p (o w u v) -> w p o u v", o=O, w=NW, u=NU, v=V
                        )[wdw, :, :, first_u:NU]
                        in1 = pw[:].rearrange(
                            "p (u o v) -> p o u v", u=DU, o=O, v=V
                        )
                        in2 = pre[:].rearrange("p (u o) -> p o u", u=DU)
                        in2 = in2.unsqueeze(3).broadcast_to([P, O, DU, V])
                        nc.vector.tensor_add(out4, in1, in2)
                        if wdw == 0:
                            # ---- bias fill for s < fill_s, all windows (ACT)
                            outf = out_sb[:].rearrange(
                                "p (o w s) -> p o w s", o=O, w=NW
                            )[:, :, :, 0:fill_s]
                            nc.scalar.copy(
                                outf,
                                bfill[:].rearrange("p (o s) -> p o s", o=O)
                                .unsqueeze(2).broadcast_to([P, O, NW, fill_s]),
                            )
                        if wdw == NW - 1:
                            nc.scalar.dma_start(
                                yv[st],
                                out_sb[:].rearrange("p (o hs) -> p o hs", o=O),
                            )

                    pending.append(stage_b)
            for fn in pending:
                fn()
    nc.compile()
    return nc


def _host_constants(weight, bias, n_discard, n_keep):
    S = n_discard + n_keep
    assert S == NU * V
    w = weight.reshape(O, C, n_keep).transpose(2, 1, 0)  # (n_keep, C, O)
    w_full = np.concatenate(
        [np.zeros((n_discard, C, O), np.float32), w.astype(np.float32)], axis=0
    )  # (S, C, O)
    act = [u for u in range(NU)
           if np.abs(w_full[u * V:(u + 1) * V]).max() > 0]
    # kernel assumes active blocks are trailing & contiguous
    first_u = act[0] if act else NU
    assert act == list(range(first_u, NU))
    DU = len(act)
    fill_s = first_u * V
    bf16 = mybir.dt.np(BF16)
    w_all = np.zeros((P, DU * 128), np.float32)
    for idx, u in enumerate(act):
        blk = w_full[u * V:(u + 1) * V]  # (V, C, O)
        # Wtri: k=(c,vp) -> n=(o,v), vp <= v
        tri = np.zeros((C, V, O, V), np.float32)
        for v in range(V):
            for vp in range(v + 1):
                tri[:, vp, :, v] = blk[vp]
        w_all[:, idx * 128:(idx + 1) * 128] = tri.reshape(C * V, O * V)
    bias32 = bias.astype(np.float32)
    consts = {
        "w_all": np.ascontiguousarray(w_all).astype(bf16),
        "bias16": np.ascontiguousarray(
            bias32[None, :] * np.ones((P, 1), np.float32)
        ),
        "ident": np.eye(P, dtype=np.float32).astype(bf16),
        "bias_fill": np.ascontiguousarray(
            np.tile(bias32[:, None], (1, fill_s)).reshape(1, -1)
            * np.ones((P, 1), np.float32)
        ).astype(bf16),
    }
    return consts, DU


def _run(inputs, trace=False):
    x = np.asarray(inputs["x"], dtype=np.float32)
    weight = np.asarray(inputs["weight"], dtype=np.float32)
    bias = np.asarray(inputs["bias"], dtype=np.float32)
    n_discard = int(inputs["n_discard"])
    n_keep = int(inputs["n_keep"])
    assert x.shape == (B, C, T) and weight.shape == (O, C * n_keep)

    consts, DU = _host_constants(weight, bias, n_discard, n_keep)
    key = ("nc", DU)
    if key not in _cache:
        _cache[key] = _build_nc(DU)
    nc = _cache[key]

    in_maps = []
    for b in range(B):
        m = dict(consts)
        m["x"] = np.ascontiguousarray(x[b])
        in_maps.append(m)
    res = run_bass_kernel_spmd(nc, in_maps, list(range(B)), trace=trace)
    y = np.stack(
        [res.results[b]["y"].astype(np.float32) for b in range(B)], axis=0
    )
    return y, res


def kernel(**inputs):
    y, _ = _run(inputs, trace=False)
    return y


# revision 19
# speedup vs baseline: 1.5322x; 1.0029x over previous
"""Trainium2 Bass kernel for nn_CumulativeFlattenedLinear (segment_reduce).

Computation: per window of S=64 timesteps, per-timestep C->O linear projection
(weights zero for the first n_discard steps) followed by a causal cumsum within
the window, plus bias.

Strategy (data-parallel over batch, 1 batch element per core):
  - x loaded f32 via HWDGE (sync queue), partition = 256-element time chunk,
    1KB contiguous DRAM runs; supertile 0 split in halves to start earlier.
  - Per window: DVE gathers the active sub-blocks into (u, c, v) column order,
    casting f32->bf16; per 8-step sub-block u the 128x128 block is transposed
    on the TensorEngine (bf16: 1 cyc/row), then one bf16 matmul per sub-block
    against a host-built triangular weight block (columns ordered (o, v))
    computes the intra-block causal cumsum of projections: pw[p, (o,v)] PSUM.
  - Cross-sub-block prefix: ACT copies the five block totals (v=7 lanes) to
    SBUF, GpSimd chains them with the bias seed; one strided DVE add per
    window combines intra + prefix into the (o, t)-ordered bf16 output tile;
    ACT fills the discarded head with bias.
  - y is stored as bf16 (rel err ~4e-3 << 2e-2 gate) and upcast to f32 on
    host; HBM traffic is 12MB/core (8 in + 4 out) ~= 34us at 358 GB/s.
"""
import numpy as np

import concourse.bass as bass
import concourse.tile as tile
from concourse import bacc, mybir
from concourse.bass_utils import run_bass_kernel_spmd

F32 = mybir.dt.float32
BF16 = mybir.dt.bfloat16

# problem geometry (asserted against inputs at runtime)
B, C, T, O = 8, 16, 131072, 16
P = 128
CH = 256                 # time-elements per partition per supertile
NST = T // (P * CH)      # 4 supertiles
V = 8                    # sub-block length
NU = 8                   # sub-blocks per window

_cache = {}


def _build_nc(du_count):
    """Build the per-core Bass program. du_count = number of active sub-blocks
    (those with any nonzero weight), assumed to be the trailing ones."""
    S = NU * V  # 64
    NW = CH // S  # windows per partition = 4
    DU = du_count
    first_u = NU - DU          # first active sub-block
    fill_s = first_u * V       # s < fill_s -> output = bias

    nc = bacc.Bacc("TRN2", target_bir_lowering=False, debug=False)
    x_d = nc.dram_tensor("x", (C, T), F32, kind="ExternalInput")
    w_d = nc.dram_tensor("w_all", (P, DU * 128), BF16, kind="ExternalInput")
    b16_d = nc.dram_tensor("bias16", (P, O), F32, kind="ExternalInput")
    ident_d = nc.dram_tensor("ident", (P, P), BF16, kind="ExternalInput")
    bfill_d = nc.dram_tensor("bias_fill", (P, O * fill_s), BF16,
                             kind="ExternalInput")
    y_d = nc.dram_tensor("y", (O, T), BF16, kind="ExternalOutput")

    xv = x_d.ap().rearrange("c (st p hs) -> st p c hs", st=NST, p=P, hs=CH)
    yv = y_d.ap().rearrange("o (st p hs) -> st p o hs", st=NST, p=P, hs=CH)

    with tile.TileContext(nc) as tc:
        SKEW = 3
        with (
            tc.tile_pool(name="const", bufs=1) as cp,
            tc.tile_pool(name="io", bufs=3) as io,
            tc.tile_pool(name="mid", bufs=SKEW + 2) as mid,
            tc.tile_pool(name="psT", bufs=2, space="PSUM") as psT,
            tc.tile_pool(name="psW", bufs=3, space="PSUM") as psW,
        ):
            xins = {}
            CQ = 4          # channels per load chunk: 4 DMAs per supertile

            def prefetch(st, parts):
                if st not in xins:
                    xins[st] = io.tile([P, C * CH], F32, name="xin", tag="xin")
                xr = xins[st][:].rearrange("p (c hs) -> p c hs", c=C)
                for q in parts:
                    nc.sync.dma_start(
                        xr[:, q * CQ:(q + 1) * CQ],
                        xv[st][:, q * CQ:(q + 1) * CQ],
                    )

            # first supertile load chunks go out before the consts
            prefetch(0, range(C // CQ))

            w_all = cp.tile([P, DU * 128], BF16, name="w_all")
            nc.scalar.dma_start(w_all[:], w_d.ap())
            bias16 = cp.tile([P, O], F32, name="bias16")
            nc.scalar.dma_start(bias16[:], b16_d.ap())
            ident = cp.tile([P, P], BF16, name="ident")
            nc.scalar.dma_start(ident[:], ident_d.ap())
            bfill = cp.tile([P, O * fill_s], BF16, name="bfill")
            nc.scalar.dma_start(bfill[:], bfill_d.ap())
            # statically-seeded prefix tiles (col 0:O = bias, never rewritten)
            pre_tiles = []
            for k in range(4):
                pre = cp.tile([P, DU * O], F32, name=f"pre{k}")
                nc.vector.tensor_copy(pre[:, 0:O], bias16[:])
                pre_tiles.append(pre)

            if NST > 1:
                prefetch(1, range(C // CQ))

            pending = []

            for st in range(NST):
                if st + 2 < NST:
                    prefetch(st + 2, range(C // CQ))
                xin = xins.pop(st)
                out_sb = io.tile([P, O * CH], BF16, name="out_sb", tag="out")
                for wdw in range(NW):
                    # ---- stage A: shuffle (DVE, f32->bf16 cast)
                    #      + transposes (PE) + PSUM->SBUF copy (ACT)
                    shuf = mid.tile([P, DU * 128], BF16, name="shuf",
                                    tag="shuf")
                    src = xin[:].rearrange(
                        "p (c w u v) -> w p u c v", c=C, w=NW, u=NU, v=V
                    )[wdw, :, first_u:NU]
                    nc.vector.tensor_copy(
                        shuf[:].rearrange("p (u c v) -> p u c v",
                                          u=DU, c=C, v=V),
                        src,
                    )
                    pt = psT.tile([P, DU * 128], BF16, name="pt", tag="pt")
                    for i in range(DU):
                        nc.tensor.transpose(
                            pt[:, i * 128:(i + 1) * 128],
                            shuf[:, i * 128:(i + 1) * 128],
                            ident[:],
                            tile_position=(0, 0),
                        )
                    ts = mid.tile([P, DU * 128], BF16, name="ts", tag="ts")
                    nc.scalar.copy(ts[:], pt[:])

                    if len(pending) >= SKEW:
                        pending.pop(0)()

                    def stage_b(st=st, wdw=wdw, ts=ts, out_sb=out_sb):
                        # ---- matmuls: intra-block triangular projections
                        #      pw columns ordered (u, o, v)
                        pw = psW.tile([P, DU * 128], F32, name="pw", tag="pw")
                        for i in range(DU):
                            nc.tensor.matmul(
                                pw[:, i * 128:(i + 1) * 128],
                                ts[:, i * 128:(i + 1) * 128],
                                w_all[:, i * 128:(i + 1) * 128],
                                start=True,
                                stop=True,
                            )
                        # ---- block totals (v = V-1 lanes) -> SBUF (ACT)
                        tot = mid.tile([P, (DU - 1) * O], F32, name="tot",
                                       tag="tot")
                        nc.scalar.copy(
                            tot[:].rearrange("p (u o) -> p u o", u=DU - 1),
                            pw[:].rearrange(
                                "p (u o v) -> p u o v", u=DU, o=O, v=V
                            )[:, 0:DU - 1, :, V - 1],
                        )
                        # ---- prefix chain with bias seed (GpSimd)
                        pre = pre_tiles[(st * NW + wdw) % 4]
                        for i in range(1, DU):
                            nc.gpsimd.tensor_add(
                                pre[:, i * O:(i + 1) * O],
                                pre[:, (i - 1) * O:i * O],
                                tot[:, (i - 1) * O:i * O],
                            )
                        # ---- combine: out[(o, s)] = intra + pre_bcast (DVE)
                        out4 = out_sb[:].rearrange(
                            "p (o w u v) -> w p o u v", o=O, w=NW, u=NU, v=V
                        )[wdw, :, :, first_u:NU]
                        in1 = pw[:].rearrange(
                            "p (u o v) -> p o u v", u=DU, o=O, v=V
                        )
                        in2 = pre[:].rearrange("p (u o) -> p o u", u=DU)
                        in2 = in2.unsqueeze(3).broadcast_to([P, O, DU, V])
                        nc.vector.tensor_add(out4, in1, in2)
                        if wdw == 0:
                            # ---- bias fill for s < fill_s, all windows (ACT)
                            outf = out_sb[:].rearrange(
                                "p (o w s) -> p o w s", o=O, w=NW
                            )[:, :, :, 0:fill_s]
                            nc.scalar.copy(
                                outf,
                                bfill[:].rearrange("p (o s) -> p o s", o=O)
                                .unsqueeze(2).broadcast_to([P, O, NW, fill_s]),
                            )
                        if wdw == NW - 1:
                            nc.scalar.dma_start(
                                yv[st],
                                out_sb[:].rearrange("p (o hs) -> p o hs", o=O),
                            )

                    pending.append(stage_b)
            for fn in pending:
                fn()
    nc.compile()
    return nc


def _host_constants(weight, bias, n_discard, n_keep):
    S = n_discard + n_keep
    assert S == NU * V
    w = weight.reshape(O, C, n_keep).transpose(2, 1, 0)  # (n_keep, C, O)
    w_full = np.concatenate(
        [np.zeros((n_discard, C, O), np.float32), w.astype(np.float32)], axis=0
    )  # (S, C, O)
    act = [u for u in range(NU)
           if np.abs(w_full[u * V:(u + 1) * V]).max() > 0]
    # kernel assumes active blocks are trailing & contiguous
    first_u = act[0] if act else NU
    assert act == list(range(first_u, NU))
    DU = len(act)
    fill_s = first_u * V
    bf16 = mybir.dt.np(BF16)
    w_all = np.zeros((P, DU * 128), np.float32)
    for idx, u in enumerate(act):
        blk = w_full[u * V:(u + 1) * V]  # (V, C, O)
        # Wtri: k=(c,vp) -> n=(o,v), vp <= v
        tri = np.zeros((C, V, O, V), np.float32)
        for v in range(V):
            for vp in range(v + 1):
                tri[:, vp, :, v] = blk[vp]
        w_all[:, idx * 128:(idx + 1) * 128] = tri.reshape(C * V, O * V)
    bias32 = bias.astype(np.float32)
    consts = {
        "w_all": np.ascontiguousarray(w_all).astype(bf16),
        "bias16": np.ascontiguousarray(
            bias32[None, :] * np.ones((P, 1), np.float32)
        ),
        "ident": np.eye(P, dtype=np.float32).astype(bf16),
        "bias_fill": np.ascontiguousarray(
            np.tile(bias32[:, None], (1, fill_s)).reshape(1, -1)
            * np.ones((P, 1), np.float32)
        ).astype(bf16),
    }
    return consts, DU


def _run(inputs, trace=False):
    x = np.asarray(inputs["x"], dtype=np.float32)
    weight = np.asarray(inputs["weight"], dtype=np.float32)
    bias = np.asarray(inputs["bias"], dtype=np.float32)
    n_discard = int(inputs["n_discard"])
    n_keep = int(inputs["n_keep"])
    assert x.shape == (B, C, T) and weight.shape == (O, C * n_keep)

    consts, DU = _host_constants(weight, bias, n_discard, n_keep)
    key = ("nc", DU)
    if key not in _cache:
        _cache[key] = _build_nc(DU)
    nc = _cache[key]

    in_maps = []
    for b in range(B):
        m = dict(consts)
        m["x"] = np.ascontiguousarray(x[b])
        in_maps.append(m)
    res = run_bass_kernel_spmd(nc, in_maps, list(range(B)), trace=trace)
    y = np.stack(
        [res.results[b]["y"].astype(np.float32) for b in range(B)], axis=0
    )
    return y, res


def kernel(**inputs):
    y, _ = _run(inputs, trace=False)
    return y
